# revision 1
# baseline (speedup 1.0000x reference)
"""Cosine-similarity (pairwise, normalized by sqrt(|a||b|)+eps) Trainium2 kernel.

Problem: first_vector [8192, 512] f32, second_vector [8192, 512] f32,
output sim [8192, 8192] f32 with
    sim = (A @ B.T) / (sqrt(|A_n| * |B_m|) + 1e-6)        (normalize=1)

Strategy (8 NeuronCores, SPMD, no collectives):
  * 2D shard: 4-way over A rows x 2-way over B rows. Core c=(ni,mj)
    computes the [2048, 4096] output slab at (ni*2048, mj*4096).
  * fp16 operands (the GEMM runs at the fp16 PE roofline, ~215ns per
    128x128x512 matmul; fp32 is 1/4 rate) and fp16 output stores (upcast
    to f32 on host). The all-f32 baseline was DMA-bound: 46MB at
    ~300GB/s/core = 154us > ~122us of PE work.
  * Inputs are packed host-side into a [*, 128, 1024] tile-pair layout:
    partition p of pair tp holds rows tp*256+p and tp*256+128+p
    back-to-back (two natural [128, 512] row-tiles side by side). This
    keeps 2KB contiguous per partition line - f16 tiles loaded from the
    row-major layout have 1KB lines, which run the DMA engines at half
    throughput (~190GB/s measured vs ~300+).
  * Both operands are pre-scaled by ssq^-1/4 on device (fused f16
    multiply) so the GEMM output is already normalized and PSUM
    evacuation is a plain 2-bank cast. (Folding a scale into the
    evacuation instead costs ~+220ns on every [128,1024] evacuation -
    measured - which is more than the 48 prep-side multiplies.)
  * Transposes to d-major are PE matmuls against an f16 identity (~56ns
    warm, hidden in the GEMM stream).
  * The non-PE work (~8.4M PSUM f32 elements to evacuate at ~1.1ns/elem
    per engine, 48 row sums-of-squares, 48 scale multiplies, transpose
    casts) totals ~165us against two engines whose window is the ~122us
    GEMM - so every balanceable op is dispatched to whichever of DVE/ACT
    has the least issued work (static greedy balance at build time, with
    measured per-op costs). Sums-of-squares run as tensor_tensor square
    + tensor_reduce on DVE (tensor_tensor_reduce would fuse them but
    crashes the exec unit on this silicon - probed) and as
    Square-activation+accumulate on ACT.
  * ACT tables (Square/Copy -> 0, Sqrt -> 1) are preloaded via dummy ops
    at t=0 (they otherwise lazy-load 1.28us mid-chain), and warm-up
    matmuls run during the input-DMA wait so the PE's HAM clock gate is
    already 8/8 when the real stream begins.
"""

import numpy as np

_N, _M, _D = 8192, 8192, 512
_P = 128
_GRID_N, _GRID_M = 4, 2
_AN = _N // _GRID_N        # A rows per core (2048)
_BM = _M // _GRID_M        # B rows per core (4096)
_KC = _D // _P             # contraction chunks (4)
_NS = 512                  # moving free dim per matmul (one PSUM bank of f32)

TRACE = False              # test harness sets True to collect an NTFF profile
LAST_RESULTS = None        # BassKernelResults of the last run (for test.py)

_NC_CACHE = {}


def _build_nc(normalize: bool):
    import concourse.bass as bass
    import concourse.mybir as mybir
    import concourse.tile as tile
    from concourse import bacc
    from concourse.masks import make_identity

    f32 = mybir.dt.float32
    f16 = mybir.dt.float16
    nc = bacc.Bacc("TRN2", target_bir_lowering=False, debug=False,
                   enable_asserts=False)

    KA = _AN // _P             # 16 A row-tiles (8 pairs)
    NSC = _BM // _NS           # 8 B column groups of 512 (2 pairs each)
    SQ = mybir.ActivationFunctionType.Square
    CP = mybir.ActivationFunctionType.Copy
    MUL = mybir.AluOpType.mult
    ADD = mybir.AluOpType.add

    # Tile-pair packed inputs (see module docstring).
    a_d = nc.declare_dram_parameter("a", [_AN // 2, 2 * _D], f16, isOutput=False)
    b_d = nc.declare_dram_parameter("b", [_BM // 2, 2 * _D], f16, isOutput=False)
    out_d = nc.declare_dram_parameter("out", [_AN, _BM], f16, isOutput=True)

    # Greedy DVE/ACT balance: issued-work counters (ns, measured costs).
    load = {"dve": 0.0, "act": 0.0}

    def lighter():
        return "dve" if load["dve"] <= load["act"] else "act"

    with tile.TileContext(nc) as tc:
        with (
            tc.tile_pool(name="const", bufs=1) as const_pool,
            tc.tile_pool(name="persist", bufs=1) as persist,
            tc.tile_pool(name="natp", bufs=6) as natp,
            tc.tile_pool(name="scaledp", bufs=6) as scaledp,
            tc.tile_pool(name="sqp", bufs=2) as sqp,
            tc.tile_pool(name="scal", bufs=6) as scal,
            tc.tile_pool(name="tpa", bufs=2, space=bass.MemorySpace.PSUM) as tpa,
            tc.tile_pool(name="tpb", bufs=1, space=bass.MemorySpace.PSUM) as tpb,
            tc.tile_pool(name="mpsum", bufs=2, space=bass.MemorySpace.PSUM) as mpsum,
        ):
            # ACT table preloads on dependency-free data, overlapping the
            # first input DMAs.
            dsrc = const_pool.tile([_P, 1], f32)
            nc.vector.memset(dsrc[:], 1.0)
            ddst = const_pool.tile([_P, 1], f32)
            nc.scalar.activation(ddst[:], dsrc[:], SQ)
            nc.scalar.sqrt(ddst[:], dsrc[:])

            # Warm-up stream source (no GpSimd dep, unlike the identity).
            wsrc = const_pool.tile([_P, _NS], f16)
            nc.vector.memset(wsrc[:], 0.5)

            ident = const_pool.tile([_P, _P], f16)
            make_identity(nc, ident[:])

            # d-major (transposed) scaled f16 operands, one tile per A
            # row-tile / per B column group for fine GEMM dependencies.
            aTt = [persist.tile([_P, _KC * _P], f16, name=f"aT{t}", tag=f"aT{t}")
                   for t in range(KA)]                       # 16 x 128 KB
            bTs = [persist.tile([_P, _KC, _NS], f16, name=f"bS{s}", tag=f"bS{s}")
                   for s in range(NSC)]                      # 8 x 512 KB

            # Output staging: 8 slots x 2 row-tiles x 1024 cols (f16).
            ostP = [persist.tile([_P, 2, 2 * _NS], f16, name=f"ost{i}",
                                 tag=f"ost{i}")
                    for i in range(KA // 2)]

            # Warm the PE / HAM clock gate during the input-DMA wait.
            # (tag="pt": pools key buffer slots by tag, which defaults to
            # the assignee name - an own tag would cost an extra bank.)
            warm = tpa.tile([_P, _KC * _P], f32, tag="pt")
            for _ in range(2):
                nc.tensor.matmul(warm[:], lhsT=ident[:], rhs=wsrc[:],
                                 start=True, stop=True)
            warm2 = tpa.tile([_P, _KC * _P], f32, tag="pt")
            for _ in range(2):
                nc.tensor.matmul(warm2[:], lhsT=ident[:], rhs=wsrc[:],
                                 start=True, stop=True)

            def ssq_pair(nat2, ssq2):
                """ssq2 (a [128, 2] slice) = row sums-of-squares of a
                [128, 2, 512] f16 tile-pair, on the lighter engine."""
                if lighter() == "dve":
                    sq2 = sqp.tile([_P, 2, _D], f16, tag="sq")
                    nc.vector.tensor_tensor(sq2[:], nat2[:], nat2[:], op=MUL)
                    nc.vector.tensor_reduce(ssq2, sq2[:],
                                            axis=mybir.AxisListType.X, op=ADD)
                    load["dve"] += 2150
                else:
                    for j in range(2):
                        sq = sqp.tile([_P, _D], f16, tag="sq")
                        nc.scalar.activation(sq[:], nat2[:, j], SQ,
                                             accum_out=ssq2[:, j:j + 1])
                    load["act"] += 1970

            def scale_tile(scaled, nat, s1):
                """scaled = nat * s1 (f16, per-partition scalar)."""
                if lighter() == "dve":
                    nc.vector.tensor_scalar_mul(scaled, in0=nat, scalar1=s1)
                    load["dve"] += 330
                else:
                    nc.scalar.activation(scaled, nat, CP, scale=s1)
                    load["act"] += 700

            def tevac(dst, ps, cost=(700, 760)):
                """Transpose-PSUM -> SBUF cast on the lighter engine."""
                if lighter() == "dve":
                    nc.vector.tensor_copy(dst, ps)
                    load["dve"] += cost[0]
                else:
                    nc.scalar.copy(dst, ps)
                    load["act"] += cost[1]

            def chain(ssqn, dst):
                """dst = ssq^-1/4 for a [128, nj] tile of row ssq."""
                nj = ssqn.shape[1]
                rec = scal.tile([_P, nj], f32, tag="rec")
                nc.vector.reciprocal(rec[:], ssqn[:])
                load["dve"] += 180
                sh = scal.tile([_P, nj], f32, tag="sh")
                nc.scalar.sqrt(sh[:], rec[:])
                nc.scalar.sqrt(dst, sh[:])
                load["act"] += 580

            def prep(src_d, pairs, finish):
                """Load tile-pairs, sums-of-squares, ssq^-1/4 chain, scale
                (f16), then hand the 4 scaled row-tiles to `finish`."""
                nats = []
                ssq4 = (scal.tile([_P, 4], f32, name="ssq4", tag="ssq")
                        if normalize else None)
                for i, tp in enumerate(pairs):
                    nat2 = natp.tile([_P, 2, _D], f16, tag="nat")
                    nc.sync.dma_start(nat2[:], src_d[tp * _P:(tp + 1) * _P, :]
                                      .rearrange("p (j d) -> p j d", j=2))
                    nats.append(nat2)
                    if normalize:
                        ssq_pair(nat2, ssq4[:, 2 * i:2 * i + 2])
                if not normalize:
                    finish([nats[j // 2][:, j % 2] for j in range(4)])
                    return
                s4 = scal.tile([_P, 4], f32, tag="s4")
                chain(ssq4[:], s4[:])
                scl = []
                for j in range(4):
                    scaled = scaledp.tile([_P, _D], f16, tag="scaled")
                    scale_tile(scaled[:], nats[j // 2][:, j % 2],
                               s4[:, j:j + 1])
                    scl.append(scaled[:])
                finish(scl)

            def prep_a(g):
                """A tile-pairs 2g, 2g+1 (row-tiles 4g..4g+3) -> aTt."""
                def finish(scl):
                    for j in range(4):
                        pt = tpa.tile([_P, _KC * _P], f32, tag="pt")
                        for k in range(_KC):
                            nc.tensor.matmul(
                                pt[:, k * _P:(k + 1) * _P],
                                lhsT=scl[j][:, k * _P:(k + 1) * _P],
                                rhs=ident[:],
                                start=True,
                                stop=True,
                            )
                        tevac(aTt[4 * g + j][:], pt[:])
                prep(a_d, (2 * g, 2 * g + 1), finish)

            def prep_b(sg):
                """B column group sg (row-tiles 4sg..4sg+3) -> bTs[sg]."""
                def finish(scl):
                    for jp in range(2):
                        ptb = tpb.tile([_P, _KC, 2 * _P], f32, tag="ptb")
                        for jj in range(2):
                            for k in range(_KC):
                                nc.tensor.matmul(
                                    ptb[:, k, jj * _P:(jj + 1) * _P],
                                    lhsT=scl[2 * jp + jj][:, k * _P:(k + 1) * _P],
                                    rhs=ident[:],
                                    start=True,
                                    stop=True,
                                )
                        tevac(bTs[sg][:, :, 2 * jp * _P:2 * (jp + 1) * _P],
                              ptb[:], cost=(1230, 1100))
                prep(b_d, (2 * sg, 2 * sg + 1), finish)

            # Fast start: shortest chain to the first GEMM matmul, then
            # backfill. DMA queues drain in emission order, so this is
            # also the input-arrival order.
            prep_a(0)
            prep_b(0)
            prep_b(1)
            prep_a(1)
            prep_a(2)
            prep_a(3)

            cidx = 0

            def evac(dst, ps):
                """Plain 2-bank PSUM -> f16 SBUF cast (operands are
                pre-normalized), alternating by measured cost."""
                nonlocal cidx
                if lighter() == "dve":
                    nc.vector.tensor_copy(dst, ps)
                    load["dve"] += 1230
                else:
                    nc.scalar.copy(dst, ps)
                    load["act"] += 1100
                cidx += 1

            def mm_ts(t, s, pdst):
                for k in range(_KC):
                    nc.tensor.matmul(
                        pdst,
                        lhsT=aTt[t][:, k * _P:(k + 1) * _P],
                        rhs=bTs[s][:, k, :],
                        start=(k == 0),
                        stop=(k == _KC - 1),
                    )

            # Column-group pairs, t-major: both B groups of the pair per
            # row-tile, one 2-bank evacuation + one 256KB store per
            # (t, pair). Remaining B prep is staggered through the loop,
            # a pair ahead of use.
            for p in range(NSC // 2):
                for t in range(KA):
                    if p == 0:
                        if t == 8:
                            prep_b(2)
                        elif t == 12:
                            prep_b(3)
                    elif p < 3:
                        if t == 0:
                            prep_b(2 * p + 2)
                        elif t == 8:
                            prep_b(2 * p + 3)
                    ps2 = mpsum.tile([_P, 2, _NS], f32, tag="ps2")
                    for h in range(2):
                        mm_ts(t, 2 * p + h, ps2[:, h])
                    tp = t // 2
                    evac(ostP[tp][:, t % 2, :], ps2[:])
                    nc.sync.dma_start(
                        out_d[t * _P:(t + 1) * _P,
                              2 * p * _NS:(2 * p + 2) * _NS],
                        ostP[tp][:, t % 2, :],
                    )

    nc.compile()
    return nc


def _get_nc(normalize: bool):
    key = bool(normalize)
    if key not in _NC_CACHE:
        _NC_CACHE[key] = _build_nc(key)
    return _NC_CACHE[key]


def _pack_pairs(x16):
    """[R, 512] f16 row-major -> [R/2, 1024] where partition-row p of
    pair tp holds rows tp*256+p and tp*256+128+p side by side."""
    r = x16.shape[0]
    return (x16.reshape(r // 256, 2, _P, _D)
            .transpose(0, 2, 1, 3)
            .reshape(r // 2, 2 * _D))


def kernel(first_vector, second_vector, normalize):
    global LAST_RESULTS
    from concourse.bass_utils import run_bass_kernel_spmd

    a = np.asarray(first_vector, dtype=np.float32).astype(np.float16)
    b = np.asarray(second_vector, dtype=np.float32).astype(np.float16)
    assert a.shape == (_N, _D) and b.shape == (_M, _D)
    norm = bool(int(np.asarray(normalize)))

    nc = _get_nc(norm)

    in_maps = []
    for c in range(_GRID_N * _GRID_M):
        ni, mj = divmod(c, _GRID_M)
        in_maps.append(
            {
                "a": _pack_pairs(a[ni * _AN:(ni + 1) * _AN]),
                "b": _pack_pairs(b[mj * _BM:(mj + 1) * _BM]),
            }
        )

    res = run_bass_kernel_spmd(
        nc, in_maps, core_ids=list(range(_GRID_N * _GRID_M)), trace=TRACE
    )
    LAST_RESULTS = res

    out = np.empty((_N, _M), dtype=np.float32)
    for c in range(_GRID_N * _GRID_M):
        ni, mj = divmod(c, _GRID_M)
        out[ni * _AN:(ni + 1) * _AN, mj * _BM:(mj + 1) * _BM] = \
            res.results[c]["out"].astype(np.float32)
    return out



# revision 10
# speedup vs baseline: 1.1221x; 1.1221x over previous
"""Cosine-similarity (pairwise, normalized by sqrt(|a||b|)+eps) Trainium2 kernel.

Problem: first_vector [8192, 512] f32, second_vector [8192, 512] f32,
output sim [8192, 8192] f32 with
    sim = (A @ B.T) / (sqrt(|A_n| * |B_m|) + 1e-6)        (normalize=1)

Strategy (8 NeuronCores, SPMD, no collectives):
  * 2D shard: 4-way over A rows x 2-way over B rows. Core c=(ni,mj)
    computes the [2048, 4096] output slab at (ni*2048, mj*4096).
  * All matmul operands are packed HOST-side into d-major (transposed)
    tiled layouts, so the PE does nothing but the 512 GEMM matmuls
    (~215ns each at the fp16 roofline) plus 8 cheap sum-of-squares
    matmuls. The previous design transposed on-device via identity
    matmuls (+15us PE) and burned ~55us of DVE/ACT on transpose
    evacuations + operand scaling.
  * Normalization is separable (eps shifts the result by <1e-7 rel):
    scale A rows by ssqA^-1/4 and B rows by ssqB^-1/4.
      - A ssq: an auxiliary ROW-major copy of A is loaded (+2MB DMA);
        square-with-accum_out gives ssq in column form [128,1] per
        row-tile - exactly the per-partition scalar an evacuation wants.
      - B ssq: square+add the d-major tiles, then one matmul against an
        all-ones [128,128] lhsT: out[p,f] = sum_k s1[k,f] for every p -
        reduce AND broadcast in one ~215ns PE op (the probed gpsimd
        partition_all_reduce costs 3537ns).
      - chains run reciprocal_approx_fast (DVE, ~51 ULP, straight off
        PSUM for B) then sqrt+sqrt on ACT, landing sbB in f16
        broadcast form / saT in f32 column form.
  * Evacuations (64 x [128,2,512] PSUM f32 -> f16), emitted 2 slots
    behind the GEMM (so every chain producer precedes its consumer in
    each engine's program order; the PSUM pool holds 3 slots):
      - column-group pair p=0 runs on RAW B operands; DVE
        scalar_tensor_tensor applies BOTH scales in one ~1300ns op:
        out = (psum * saT[t]) * sbB[0:2]. This keeps the B-scale chain
        off the first-GEMM critical path (first real matmul ~2us after
        the instruction-load prolog, when the first DMAs land).
      - groups >= 2 are pre-scaled on GpSimd (otherwise idle) and
        evacuated with a per-partition scale only: DVE tensor_scalar
        or ACT activation-with-scale, greedy-balanced by measured cost.
  * ACT tables (Sqrt then Square) preload on dummy data at t=0;
    warm-up matmuls run during the input-DMA wait so the PE clock
    is ramped when the real stream begins.
  * fp16 everywhere off-chip: 8MB in, 16.8MB out per core, against a
    ~112us PE window - DMA never binds.
"""

import numpy as np

_N, _M, _D = 8192, 8192, 512
_P = 128
_GRID_N, _GRID_M = 4, 2
_AN = _N // _GRID_N        # A rows per core (2048)
_BM = _M // _GRID_M        # B rows per core (4096)
_KC = _D // _P             # contraction chunks (4)
_NS = 512                  # moving free dim per matmul (one PSUM bank of f32)

TRACE = False              # test harness sets True to collect an NTFF profile
LAST_RESULTS = None        # BassKernelResults of the last run (for test.py)

_NC_CACHE = {}


def _build_nc(normalize: bool):
    import concourse.bass as bass
    import concourse.mybir as mybir
    import concourse.tile as tile
    from concourse import bacc

    f32 = mybir.dt.float32
    f16 = mybir.dt.float16
    nc = bacc.Bacc("TRN2", target_bir_lowering=False, debug=False,
                   enable_asserts=False)

    KA = _AN // _P             # 16 A row-tiles (8 pairs)
    NSC = _BM // _NS           # 8 B column groups of 512
    NG = KA // 2               # 8 A pairs
    SQ = mybir.ActivationFunctionType.Square
    CP = mybir.ActivationFunctionType.Copy
    MUL = mybir.AluOpType.mult
    ADD = mybir.AluOpType.add

    # d-major A pairs: row (g*128+p), col (h, kc, r) = A[256g+128h+r, 128kc+p]
    ad_d = nc.declare_dram_parameter("ad", [NG * _P, 2 * _KC * _P], f16,
                                     isOutput=False)
    # d-major B groups: row (s*128+p), col (kc, c) = B[512s+c, 128kc+p]
    bd_d = nc.declare_dram_parameter("bd", [NSC * _P, _KC * _NS], f16,
                                     isOutput=False)
    # row-major A pairs (aux, for ssq): row (g*128+p), col (h, d)
    aa_d = nc.declare_dram_parameter("aa", [NG * _P, 2 * _D], f16,
                                     isOutput=False)
    out_d = nc.declare_dram_parameter("out", [_AN, _BM], f16, isOutput=True)

    # Greedy DVE/ACT balance for the sa-only evacuations (ns, measured).
    load = {"dve": 0.0, "act": 0.0}

    with tile.TileContext(nc) as tc:
        with (
            tc.tile_pool(name="const", bufs=1) as const_pool,
            tc.tile_pool(name="persist", bufs=1) as persist,
            tc.tile_pool(name="sqp", bufs=2) as sqp,
            tc.tile_pool(name="s1p", bufs=2) as s1p,
            tc.tile_pool(name="chp", bufs=2) as chp,
            tc.tile_pool(name="dmp", bufs=2) as dmp,
            tc.tile_pool(name="ostp", bufs=8) as ostp,
            tc.tile_pool(name="mpsum", bufs=3, space=bass.MemorySpace.PSUM) as mpsum,
            tc.tile_pool(name="ssqp", bufs=2, space=bass.MemorySpace.PSUM) as ssqp,
        ):
            # Memsets first so every engine's first data arrives ASAP.
            wsrc = const_pool.tile([_P, _NS], f16)
            nc.vector.memset(wsrc[:], 0.5)
            ones = const_pool.tile([_P, _P], f16)
            nc.vector.memset(ones[:], 1.0)
            dsrc = const_pool.tile([_P, 1], f32)
            nc.vector.memset(dsrc[:], 1.0)

            # ACT table preloads on dependency-free data (Sqrt is needed
            # first, by the chains at ~6us; Square by A/B squares later).
            ddst = const_pool.tile([_P, 1], f32)
            nc.scalar.sqrt(ddst[:], dsrc[:])
            nc.scalar.activation(ddst[:], dsrc[:], SQ)

            # Persistent operand tiles.
            adT = [persist.tile([_P, 2, _KC, _P], f16, name=f"ad{g}", tag=f"ad{g}")
                   for g in range(NG)]                     # 8 x 256 KB
            bdT = [persist.tile([_P, _KC, _NS], f16, name=f"bd{s}", tag=f"bd{s}")
                   for s in range(NSC)]                    # 8 x 512 KB
            bdS = [None, None] + \
                  [persist.tile([_P, _KC, _NS], f16, name=f"bs{s}", tag=f"bs{s}")
                   for s in range(2, NSC)]                 # 6 x 512 KB (scaled)
            aaT = [persist.tile([_P, 2, _D], f16, name=f"aa{g}", tag=f"aa{g}")
                   for g in range(NG)]                     # 8 x 256 KB

            ssqA = persist.tile([_P, KA], f32, name="ssqA", tag="ssqA")
            shA = persist.tile([_P, KA], f32, name="shA", tag="shA")
            saT = persist.tile([_P, KA], f32, name="saT", tag="saT")
            sbB = persist.tile([_P, NSC, _NS], f16, name="sbB", tag="sbB")

            # Input DMA, fastest-needed first (queues drain in emission
            # order): B0, A0 d-major feed the first GEMM slot ~2us in.
            def dma_bd(s):
                nc.sync.dma_start(
                    bdT[s][:],
                    bd_d[s * _P:(s + 1) * _P, :]
                    .rearrange("p (k c) -> p k c", k=_KC))

            def dma_ad(g):
                nc.sync.dma_start(
                    adT[g][:],
                    ad_d[g * _P:(g + 1) * _P, :]
                    .rearrange("p (h k r) -> p h k r", h=2, k=_KC))

            def dma_aa(g):
                nc.sync.dma_start(
                    aaT[g][:],
                    aa_d[g * _P:(g + 1) * _P, :]
                    .rearrange("p (h d) -> p h d", h=2))

            dma_bd(0)
            dma_ad(0)
            dma_bd(1)
            dma_ad(1)
            dma_aa(0)
            dma_aa(1)
            dma_ad(2)
            dma_ad(3)
            dma_aa(2)
            dma_aa(3)
            dma_ad(4)
            dma_ad(5)
            dma_aa(4)
            dma_aa(5)
            dma_ad(6)
            dma_ad(7)
            dma_aa(6)
            dma_aa(7)

            # Warm the PE clock during the input-DMA wait.
            for _ in range(2):
                warm = ssqp.tile([_P, _NS], f32, tag="ssqp")
                for _ in range(2):
                    nc.tensor.matmul(warm[:], lhsT=ones[:], rhs=wsrc[:],
                                     start=True, stop=True)

            def b_squares(s, engine):
                """Squares + adds of d-major B group s -> s1 [128, 512]."""
                sq = sqp.tile([_P, _KC, _NS], f16, tag="sq")
                for j in range(2):
                    if engine == "dve":
                        nc.vector.tensor_tensor(sq[:, 2 * j:2 * j + 2],
                                                bdT[s][:, 2 * j:2 * j + 2],
                                                bdT[s][:, 2 * j:2 * j + 2],
                                                op=MUL)
                        load["dve"] += 620
                    else:
                        nc.scalar.activation(sq[:, 2 * j:2 * j + 2],
                                             bdT[s][:, 2 * j:2 * j + 2], SQ)
                        load["act"] += 1060
                s2 = s1p.tile([_P, 2, _NS], f16, tag="s2")
                s1 = s1p.tile([_P, _NS], f16, tag="s1")
                nc.vector.tensor_tensor(s2[:], sq[:, 0:2], sq[:, 2:4], op=ADD)
                nc.vector.tensor_tensor(s1[:], s2[:, 0], s2[:, 1], op=ADD)
                load["dve"] += 950
                return s1

            def b_ssq_mm(s, s1):
                """All-ones matmul: ssq_b reduced + broadcast into PSUM."""
                ps = ssqp.tile([_P, _NS], f32, tag="ssqp")
                nc.tensor.matmul(ps[:], lhsT=ones[:], rhs=s1[:], start=True,
                                 stop=True)
                return ps

            def b_chain(s, ps):
                """sbB[s] = ssq^-1/4: reciprocal_approx_fast (DVE, reads
                PSUM) then sqrt+sqrt (ACT, f16 out)."""
                rec = chp.tile([_P, _NS], f32, tag="recb")
                nc.vector.reciprocal_approx_fast(rec[:], ps[:])
                nc.scalar.sqrt(rec[:], rec[:])
                nc.scalar.sqrt(sbB[:, s, :], rec[:])
                load["dve"] += 800
                load["act"] += 1300

            def a_ssq(g, engine):
                """ssq of A pair g (column form) + chain into saT."""
                for h in range(2):
                    t = 2 * g + h
                    dump = dmp.tile([_P, _D], f16, tag="dump")
                    if engine == "dve":
                        nc.vector.scalar_tensor_tensor(
                            dump[:], aaT[g][:, h], 1.0, aaT[g][:, h],
                            op0=MUL, op1=MUL, accum_out=ssqA[:, t:t + 1])
                        load["dve"] += 640
                    else:
                        nc.scalar.activation(dump[:], aaT[g][:, h], SQ,
                                             accum_out=ssqA[:, t:t + 1])
                        load["act"] += 800
                c = slice(2 * g, 2 * g + 2)
                nc.scalar.sqrt(shA[:, c], ssqA[:, c])
                nc.scalar.sqrt(shA[:, c], shA[:, c])
                nc.vector.reciprocal(saT[:, c], shA[:, c])
                load["act"] += 400
                load["dve"] += 120

            def prescale_b(s):
                """bdS[s] = bdT[s] * sbB[s] on GpSimd (otherwise idle)."""
                for k in range(_KC):
                    nc.gpsimd.tensor_tensor(bdS[s][:, k, :], bdT[s][:, k, :],
                                            sbB[:, s, :], op=MUL)

            # Pre-loop prep: B0/B1 squares on DVE (startup critical);
            # A pair 0's ssq on ACT fills its idle window before the B
            # chains' sqrts queue up, so saT[0:2] never gates the first
            # evacuation.
            if normalize:
                s1_b = {0: b_squares(0, "dve"), 1: b_squares(1, "dve")}
                ps_b = {}
                # Pair 0's ACT work only; the DVE reciprocal is emitted at
                # slot t=1 (after the B reciprocals) to keep DVE in
                # data-arrival order.
                for h in range(2):
                    dump = dmp.tile([_P, _D], f16, tag="dump")
                    nc.scalar.activation(dump[:], aaT[0][:, h], SQ,
                                         accum_out=ssqA[:, h:h + 1])
                nc.scalar.sqrt(shA[:, 0:2], ssqA[:, 0:2])
                nc.scalar.sqrt(shA[:, 0:2], shA[:, 0:2])
                load["act"] += 2000

            # Deferred-evacuation pipeline: evac for slot i is emitted
            # alongside slot i+2 so chain producers precede consumers in
            # every engine's program order. mpsum holds 3 slots.
            pending = []

            def emit_evac():
                p, t, ps2 = pending.pop(0)
                ost = ostp.tile([_P, 2, _NS], f16, tag="ost")
                if not normalize:
                    if load["dve"] <= load["act"]:
                        nc.vector.tensor_copy(ost[:], ps2[:])
                        load["dve"] += 1230
                    else:
                        nc.scalar.copy(ost[:], ps2[:])
                        load["act"] += 1100
                elif p == 0:
                    # Raw B operands; both scales in one DVE op.
                    nc.vector.scalar_tensor_tensor(
                        ost[:], ps2[:], saT[:, t:t + 1], sbB[:, 0:2, :],
                        op0=MUL, op1=MUL)
                    load["dve"] += 1300
                else:
                    sa = saT[:, t:t + 1]
                    if load["dve"] <= load["act"]:
                        nc.vector.tensor_scalar_mul(ost[:], in0=ps2[:],
                                                    scalar1=sa)
                        load["dve"] += 1300
                    else:
                        nc.scalar.activation(ost[:], ps2[:], CP, scale=sa)
                        load["act"] += 1320
                nc.sync.dma_start(
                    out_d[t * _P:(t + 1) * _P,
                          2 * p * _NS:(2 * p + 2) * _NS],
                    ost[:].rearrange("m h f -> m (h f)"),
                )

            # Per-slot prep emissions for p=0 (exec follows data arrival;
            # placement keeps every engine stream bubble-free).
            def prep_p0(t):
                if t == 0:
                    ps_b[0] = b_ssq_mm(0, s1_b[0])
                    b_chain(0, ps_b[0])
                elif t == 1:
                    ps_b[1] = b_ssq_mm(1, s1_b[1])
                    b_chain(1, ps_b[1])
                    nc.vector.reciprocal(saT[:, 0:2], shA[:, 0:2])
                    load["dve"] += 120
                elif 2 <= t <= 8:
                    a_ssq(t - 1, "dve" if t <= 3 else "act")
                if t == 3:
                    dma_bd(2)
                elif t == 4:
                    dma_bd(3)
                elif t == 5:
                    s1_b[2] = b_squares(2, "act")
                elif t == 6:
                    s1_b[3] = b_squares(3, "act")
                elif t == 7:
                    ps_b[2] = b_ssq_mm(2, s1_b[2])
                elif t == 9:
                    ps_b[3] = b_ssq_mm(3, s1_b[3])
                    b_chain(2, ps_b[2])
                    prescale_b(2)
                elif t == 11:
                    b_chain(3, ps_b[3])
                    prescale_b(3)

            def prep_p(p, t):
                sa_, sb_ = 2 * p + 2, 2 * p + 3
                if t == 0:
                    dma_bd(sa_)
                elif t == 2:
                    dma_bd(sb_)
                elif t == 4:
                    s1_b[sa_] = b_squares(sa_, "act")
                elif t == 5:
                    s1_b[sb_] = b_squares(sb_, "act")
                elif t == 6:
                    ps_b[sa_] = b_ssq_mm(sa_, s1_b[sa_])
                elif t == 7:
                    ps_b[sb_] = b_ssq_mm(sb_, s1_b[sb_])
                elif t == 8:
                    b_chain(sa_, ps_b[sa_])
                    prescale_b(sa_)
                elif t == 9:
                    b_chain(sb_, ps_b[sb_])
                    prescale_b(sb_)

            # GEMM stream: column-group pairs p, row-tiles t.
            for p in range(NSC // 2):
                for t in range(KA):
                    if len(pending) >= 2:
                        emit_evac()

                    ps2 = mpsum.tile([_P, 2, _NS], f32, tag="ps2")
                    for h in range(2):
                        s = 2 * p + h
                        rhs_tile = bdT[s] if (p == 0 or not normalize) else bdS[s]
                        for k in range(_KC):
                            nc.tensor.matmul(
                                ps2[:, h],
                                lhsT=adT[t // 2][:, t % 2, k, :],
                                rhs=rhs_tile[:, k, :],
                                start=(k == 0),
                                stop=(k == _KC - 1),
                            )
                    pending.append((p, t, ps2))

                    if normalize:
                        if p == 0:
                            prep_p0(t)
                        elif p < 3:
                            prep_p(p, t)

            while pending:
                emit_evac()

    nc.compile()
    return nc


def _get_nc(normalize: bool):
    key = bool(normalize)
    if key not in _NC_CACHE:
        _NC_CACHE[key] = _build_nc(key)
    return _NC_CACHE[key]


def _pack_ad(a16):
    """[2048, 512] f16 -> d-major pair tiles [1024, 1024]:
    row (g*128+p), col (h*512 + kc*128 + r) = A[g*256+h*128+r, kc*128+p]."""
    return (a16.reshape(8, 2, _P, _KC, _P)
            .transpose(0, 4, 1, 3, 2)
            .reshape(8 * _P, 2 * _KC * _P))


def _pack_bd(b16):
    """[4096, 512] f16 -> d-major group tiles [1024, 2048]:
    row (s*128+p), col (kc*512 + c) = B[s*512+c, kc*128+p]."""
    return (b16.reshape(8, _NS, _KC, _P)
            .transpose(0, 3, 2, 1)
            .reshape(8 * _P, _KC * _NS))


def _pack_aa(a16):
    """[2048, 512] f16 row-major pairs [1024, 1024]: partition p of pair g
    holds rows g*256+p and g*256+128+p side by side."""
    return (a16.reshape(8, 2, _P, _D)
            .transpose(0, 2, 1, 3)
            .reshape(8 * _P, 2 * _D))


def kernel(first_vector, second_vector, normalize):
    global LAST_RESULTS
    from concourse.bass_utils import run_bass_kernel_spmd

    a = np.asarray(first_vector, dtype=np.float32).astype(np.float16)
    b = np.asarray(second_vector, dtype=np.float32).astype(np.float16)
    assert a.shape == (_N, _D) and b.shape == (_M, _D)
    norm = bool(int(np.asarray(normalize)))

    nc = _get_nc(norm)

    ad = [_pack_ad(a[ni * _AN:(ni + 1) * _AN]) for ni in range(_GRID_N)]
    aa = [_pack_aa(a[ni * _AN:(ni + 1) * _AN]) for ni in range(_GRID_N)]
    bd = [_pack_bd(b[mj * _BM:(mj + 1) * _BM]) for mj in range(_GRID_M)]

    in_maps = []
    for c in range(_GRID_N * _GRID_M):
        ni, mj = divmod(c, _GRID_M)
        in_maps.append({"ad": ad[ni], "aa": aa[ni], "bd": bd[mj]})

    res = run_bass_kernel_spmd(
        nc, in_maps, core_ids=list(range(_GRID_N * _GRID_M)), trace=TRACE
    )
    LAST_RESULTS = res

    out = np.empty((_N, _M), dtype=np.float32)
    for c in range(_GRID_N * _GRID_M):
        ni, mj = divmod(c, _GRID_M)
        out[ni * _AN:(ni + 1) * _AN, mj * _BM:(mj + 1) * _BM] = \
            res.results[c]["out"].astype(np.float32)
    return out
